# revision 20
# baseline (speedup 1.0000x reference)
"""Trainium2 Bass kernel for MeshInterpolate (interpolate_face_attributes).

Problem (hardcoded shapes):
  pix_to_face [4, 512, 512, 1] int (-1 = background), values in [-1, 10000)
  bary_coords [4, 512, 512, 1, 3] f32
  face_memory [10000, 3, 128] f32
  output      [4, 128, 512, 512] f32 (NCHW)

Sharding: data-parallel over (N, H/2): 8 cores, core c handles image c//2,
rows 256*(c%2) .. +256  -> 131072 pixels per core. face_memory replicated.

v5 "sorted one-hot matmul" design:
  Host sorts each core's pixels by face id and packs them into superblocks
  of NPX=256 pixels with at most UMAX=30 unique faces (a superblock of 256
  face-sorted pixels spans ~20 faces; packing pads the rare overflow).
  For each superblock the device:
    - dma_gather's the superblock's unique (face, vertex) rows from
      fm viewed as [30000, 128] bf16 (idx = 3*face+v, 256B elements) into a
      staging tile -- k = 3u+v, k < 90, padded to 128 idxs per superblock;
    - one PE matmul  psum[c, p'] = sum_k stag[k, c] * W[k, p']  with the
      host-built weight matrix W [90, 256] bf16 carrying each pixel's three
      barycentric weights in rows 3u..3u+2 of its face's slot (exact
      per-pixel weights; zero columns for background/padding pixels).
  This replaces all per-pixel DVE/ACT multiplies of earlier versions with
  wide matmuls, cuts gather bytes ~6x (unique faces only) and descriptor
  count ~2x, and produces channel-major (NCHW) output directly.
  ACT copies psum -> SBUF bf16; sync-DMA to out; host widens bf16->f32 and
  scatters columns back to original pixel order (inverse of the sort).

  Gather descriptor generation is spread over 4 SWDGE queues (4 Q7 core
  pairs); bf16 keeps the 2e-2 tolerance with ~0.6% error (fm and W rounded
  to bf16 once; psum accumulates in f32).
"""

import os

import numpy as np

# Safety: recover wedged NeuronCores from a previous crashed process. Must be
# set before the first jax/NRT backend init in this process.
os.environ.setdefault("NEURON_RT_RESET_CORES", "1")

P = 128
C = 128               # channels
NPX = 384             # pixels per superblock
UMAX = 42             # max unique faces per superblock
KMAX = 3 * UMAX       # stationary contraction rows per superblock (126)
SB_PER_TILE = 4
TPX = SB_PER_TILE * NPX   # 2048 pixels per tile
NTILES = 87           # fixed capacity: 87*1536 = 133632 slots >= 131072+waste
NPIX_CORE = 131072
F = 10000
N_CORES = 8
NQ = 4                # SWDGE queues
CHUNK = 512           # gather idxs per call (whole tile)
NCHUNK = 1            # gather calls per tile

_CACHE = {}


def _bf16():
    import ml_dtypes

    return ml_dtypes.bfloat16


def _build_nc(ntiles=NTILES):
    import concourse.bacc as bacc
    import concourse.mybir as mybir
    from concourse import tile
    from concourse.library_config import mlp

    nc = bacc.Bacc("TRN2", target_bir_lowering=False, debug=False,
                   num_swdge_queues=NQ)
    fm3 = nc.dram_tensor("fm3", [3 * F, C], mybir.dt.bfloat16,
                         kind="ExternalInput")
    idxw = nc.dram_tensor("idxw", [ntiles, P, SB_PER_TILE * P // 16],
                          mybir.dt.int16, kind="ExternalInput")
    wmat = nc.dram_tensor("wmat", [ntiles, KMAX, SB_PER_TILE, NPX],
                          mybir.dt.bfloat16, kind="ExternalInput")
    out = nc.dram_tensor("out", [P, ntiles * TPX], mybir.dt.bfloat16,
                         kind="ExternalOutput")

    with tile.TileContext(nc) as tc:
        nc.gpsimd.load_library(mlp)
        with (
            tc.tile_pool(name="io", bufs=6) as iop,
            tc.tile_pool(name="bounce", bufs=4) as bouncep,
            tc.tile_pool(name="ps", bufs=2, space="PSUM") as psump,
        ):
            for t in range(ntiles):
                w_sb = iop.tile([KMAX, SB_PER_TILE, NPX], mybir.dt.bfloat16,
                                tag="w")
                stag_sb = iop.tile([P, SB_PER_TILE, C], mybir.dt.bfloat16,
                                   tag="stag")
                idx_sb = iop.tile([P, SB_PER_TILE * P // 16], mybir.dt.int16,
                                  tag="idx")
                nc.sync.dma_start(w_sb[:], wmat[t])
                nc.sync.dma_start(idx_sb[:], idxw[t])
                cw = CHUNK // 16
                with tc.high_priority(offset=400):
                    for ch in range(NCHUNK):
                        nc.gpsimd.dma_gather(
                            stag_sb[:, 4 * ch:4 * (ch + 1), :], fm3[:],
                            idx_sb[:, ch * cw:(ch + 1) * cw],
                            CHUNK, CHUNK, C, queue_num=t % NQ)
                ps = psump.tile([P, SB_PER_TILE, 512], mybir.dt.float32,
                                tag="ps")
                for j in range(SB_PER_TILE):
                    nc.tensor.matmul(
                        ps[:, j, 0:NPX],
                        stag_sb[0:KMAX, j, :],
                        w_sb[:, j, :],
                        start=True, stop=True,
                    )
                bounce = bouncep.tile([P, TPX], mybir.dt.bfloat16,
                                      tag="bounce")
                half = TPX // 2
                nc.scalar.copy(bounce[:, 0:half], ps[:, 0:2, 0:NPX])
                nc.vector.tensor_copy(bounce[:, half:TPX], ps[:, 2:4, 0:NPX])
                nc.scalar.dma_start(out[:, t * TPX:(t + 1) * TPX], bounce[:])
    nc.compile()
    return nc


def _get_nc():
    if "nc" not in _CACHE:
        _CACHE["nc"] = _build_nc()
    return _CACHE["nc"]


def _pack_core(idx, bary):
    """Greedy superblock packing of one core's face-sorted pixels.

    Returns (slot[131072] int64 global device slot per sorted-pixel-rank,
             idx_flat[nsb*128] int16 gather indices (3*face+v, padded),
             wmat [NTILES, KMAX, SB_PER_TILE, NPX] bf16)
    """
    bf16 = _bf16()
    bg = idx < 0
    idxc = np.where(bg, 0, idx).astype(np.int64)
    perm = np.argsort(idxc, kind="stable")       # pixel ids in face order
    sidx = idxc[perm]
    sbary = np.where(bg[perm][:, None], 0.0, bary[perm]).astype(np.float32)

    faces, counts = np.unique(sidx, return_counts=True)
    nsb_cap = NTILES * SB_PER_TILE

    # Walk face runs, assigning spans (face, u_local, sb, p0, cnt).
    span_sb = []
    span_u = []
    span_p0 = []
    span_cnt = []
    span_face = []
    sb = 0
    cur_px = 0
    cur_u = 0
    for f, cnt in zip(faces.tolist(), counts.tolist()):
        remaining = cnt
        first_in_sb = True
        u = -1
        while remaining > 0:
            if cur_px == NPX or (first_in_sb and cur_u == UMAX):
                sb += 1
                cur_px = 0
                cur_u = 0
                first_in_sb = True
            if first_in_sb:
                u = cur_u
                cur_u += 1
                first_in_sb = False
            take = min(remaining, NPX - cur_px)
            span_sb.append(sb)
            span_u.append(u)
            span_p0.append(cur_px)
            span_cnt.append(take)
            span_face.append(f)
            cur_px += take
            remaining -= take
            if remaining > 0:
                # face continues into the next superblock
                sb += 1
                cur_px = 0
                cur_u = 0
                first_in_sb = True
    nsb = sb + 1
    assert nsb <= nsb_cap, f"packing overflow: {nsb} > {nsb_cap}"

    span_sb = np.asarray(span_sb, dtype=np.int64)
    span_u = np.asarray(span_u, dtype=np.int64)
    span_p0 = np.asarray(span_p0, dtype=np.int64)
    span_cnt = np.asarray(span_cnt, dtype=np.int64)
    span_face = np.asarray(span_face, dtype=np.int64)

    # per-sorted-pixel: sb, u, p' via span expansion (spans are in sorted order)
    pix_sb = np.repeat(span_sb, span_cnt)
    pix_u = np.repeat(span_u, span_cnt)
    off_in_span = np.arange(len(sidx)) - np.repeat(
        np.cumsum(span_cnt) - span_cnt, span_cnt)
    pix_p = np.repeat(span_p0, span_cnt) + off_in_span
    slot = pix_sb * NPX + pix_p                      # global device column

    # gather index lists per superblock: row 3u+v -> 3*face+v
    sbu_face = np.zeros((nsb_cap, UMAX), dtype=np.int64)
    sbu_face[span_sb, span_u] = span_face

    idx128 = np.zeros((nsb_cap, P), dtype=np.int16)
    r = np.arange(KMAX)
    idx128[:, :KMAX] = (3 * sbu_face[:, r // 3] + (r % 3)).astype(np.int16)

    # weight matrix W[sb, 3u+v, p'] = bary_v  (exact; zero for bg/padding)
    w = np.zeros((nsb_cap, KMAX, NPX), dtype=np.float32)
    for v in range(3):
        w[pix_sb, 3 * pix_u + v, pix_p] = sbary[:, v]
    w = w.reshape(NTILES, SB_PER_TILE, KMAX, NPX).transpose(0, 2, 1, 3)
    w = np.ascontiguousarray(w).astype(bf16)

    # wrap idx streams for the gather (CHUNK=256 per call, 4 calls per tile)
    idx_flat = idx128.reshape(NTILES, NCHUNK, CHUNK)                # [nt,4,256]
    idxw = np.ascontiguousarray(
        idx_flat.reshape(NTILES, NCHUNK, CHUNK // 16, 16).transpose(0, 1, 3, 2))
    idxw = np.tile(idxw, (1, 1, 8, 1))              # [nt, 4, 128, 16]
    idxw = np.ascontiguousarray(
        idxw.transpose(0, 2, 1, 3).reshape(NTILES, P, NCHUNK * CHUNK // 16))

    return perm, slot, idxw, w


def _prep_in_maps(pix_to_face, bary_coords, face_memory):
    bf16 = _bf16()
    N, H, W_, K = pix_to_face.shape          # 4, 512, 512, 1
    assert (N, H, W_, K) == (4, 512, 512, 1)
    fm3 = np.asarray(face_memory, dtype=np.float32).reshape(3 * F, C)
    fm3 = fm3.astype(bf16)

    idx_all = np.asarray(pix_to_face).reshape(N, H, W_)
    bary_all = np.asarray(bary_coords, dtype=np.float32).reshape(N, H, W_, 3)

    in_maps = []
    maps = []
    for c in range(N_CORES):
        n, hh = c // 2, (c % 2) * 256
        idx = idx_all[n, hh:hh + 256].reshape(-1)
        bary = bary_all[n, hh:hh + 256].reshape(-1, 3)
        perm, slot, idxw, w = _pack_core(idx, bary)
        in_maps.append({"fm3": fm3, "idxw": idxw, "wmat": w})
        maps.append((perm, slot))
    _CACHE["maps"] = maps
    return in_maps


def _widen_bf16(a):
    u = np.asarray(a).view(np.uint16).astype(np.uint32) << 16
    return u.view(np.float32)


def _assemble(results, maps=None):
    maps = maps or _CACHE["maps"]
    out_full = np.empty((4, 128, 512, 512), dtype=np.float32)
    for c in range(N_CORES):
        n, hh = c // 2, (c % 2) * 256
        perm, slot = maps[c]
        dev = _widen_bf16(results[c]["out"])        # [128, NTILES*TPX]
        img = out_full[n, :, hh:hh + 256, :].reshape(128, NPIX_CORE)
        img[:, perm] = dev[:, slot]
    return out_full


def run(in_maps, trace=False, trace_kwargs=None):
    from concourse.bass_utils import run_bass_kernel_spmd

    nc = _get_nc()
    kw = {}
    if trace:
        kw = dict(trace=True, trace_kwargs=trace_kwargs or {})
    return run_bass_kernel_spmd(nc, in_maps, list(range(N_CORES)), **kw)


def kernel(pix_to_face, bary_coords, face_memory):
    in_maps = _prep_in_maps(pix_to_face, bary_coords, face_memory)
    res = run(in_maps)
    return _assemble(res.results)


# revision 21
# speedup vs baseline: 1.0723x; 1.0723x over previous
"""Trainium2 Bass kernel for MeshInterpolate (interpolate_face_attributes).

Problem (hardcoded shapes):
  pix_to_face [4, 512, 512, 1] int (-1 = background), values in [-1, 10000)
  bary_coords [4, 512, 512, 1, 3] f32
  face_memory [10000, 3, 128] f32
  output      [4, 128, 512, 512] f32 (NCHW)

Sharding: data-parallel over (N, H/2): 8 cores, core c handles image c//2,
rows 256*(c%2) .. +256  -> 131072 pixels per core. face_memory replicated.

v5 "sorted one-hot matmul" design:
  Host sorts each core's pixels by face id and packs them into superblocks
  of NPX=256 pixels with at most UMAX=30 unique faces (a superblock of 256
  face-sorted pixels spans ~20 faces; packing pads the rare overflow).
  For each superblock the device:
    - dma_gather's the superblock's unique (face, vertex) rows from
      fm viewed as [30000, 128] bf16 (idx = 3*face+v, 256B elements) into a
      staging tile -- k = 3u+v, k < 90, padded to 128 idxs per superblock;
    - one PE matmul  psum[c, p'] = sum_k stag[k, c] * W[k, p']  with the
      host-built weight matrix W [90, 256] bf16 carrying each pixel's three
      barycentric weights in rows 3u..3u+2 of its face's slot (exact
      per-pixel weights; zero columns for background/padding pixels).
  This replaces all per-pixel DVE/ACT multiplies of earlier versions with
  wide matmuls, cuts gather bytes ~6x (unique faces only) and descriptor
  count ~2x, and produces channel-major (NCHW) output directly.
  ACT copies psum -> SBUF bf16; sync-DMA to out; host widens bf16->f32 and
  scatters columns back to original pixel order (inverse of the sort).

  Gather descriptor generation is spread over 4 SWDGE queues (4 Q7 core
  pairs); bf16 keeps the 2e-2 tolerance with ~0.6% error (fm and W rounded
  to bf16 once; psum accumulates in f32).
"""

import os

import numpy as np

# Safety: recover wedged NeuronCores from a previous crashed process. Must be
# set before the first jax/NRT backend init in this process.
os.environ.setdefault("NEURON_RT_RESET_CORES", "1")

P = 128
C = 128               # channels
NPX = 384             # pixels per superblock
UMAX = 39             # max unique faces per superblock
KMAX = 3 * UMAX       # stationary contraction rows per superblock (117)
SB_PER_TILE = 4
TPX = SB_PER_TILE * NPX   # 2048 pixels per tile
NTILES = 87           # fixed capacity: 87*1536 = 133632 slots >= 131072+waste
NPIX_CORE = 131072
F = 10000
N_CORES = 8
NQ = 4                # SWDGE queues
CHUNK = 256           # gather idxs per call (2 superblocks)
NCHUNK = 2            # gather calls per tile

_CACHE = {}


def _bf16():
    import ml_dtypes

    return ml_dtypes.bfloat16


def _build_nc(ntiles=NTILES):
    import concourse.bacc as bacc
    import concourse.mybir as mybir
    from concourse import tile
    from concourse.library_config import mlp

    nc = bacc.Bacc("TRN2", target_bir_lowering=False, debug=False,
                   num_swdge_queues=NQ)
    fm3 = nc.dram_tensor("fm3", [3 * F, C], mybir.dt.bfloat16,
                         kind="ExternalInput")
    idxw = nc.dram_tensor("idxw", [ntiles, P, SB_PER_TILE * P // 16],
                          mybir.dt.int16, kind="ExternalInput")
    wmat = nc.dram_tensor("wmat", [ntiles, KMAX, SB_PER_TILE, NPX],
                          mybir.dt.bfloat16, kind="ExternalInput")
    out = nc.dram_tensor("out", [P, ntiles * TPX], mybir.dt.bfloat16,
                         kind="ExternalOutput")

    with tile.TileContext(nc) as tc:
        nc.gpsimd.load_library(mlp)
        with (
            tc.tile_pool(name="io", bufs=8) as iop,
            tc.tile_pool(name="bounce", bufs=6) as bouncep,
            tc.tile_pool(name="ps", bufs=2, space="PSUM") as psump,
        ):
            for t in range(ntiles):
                w_sb = iop.tile([KMAX, SB_PER_TILE, NPX], mybir.dt.bfloat16,
                                tag="w")
                stag_sb = iop.tile([P, SB_PER_TILE, C], mybir.dt.bfloat16,
                                   tag="stag")
                idx_sb = iop.tile([P, SB_PER_TILE * P // 16], mybir.dt.int16,
                                  tag="idx")
                nc.sync.dma_start(w_sb[:], wmat[t])
                nc.sync.dma_start(idx_sb[:], idxw[t])
                cw = CHUNK // 16
                with tc.high_priority(offset=400):
                    for ch in range(NCHUNK):
                        nc.gpsimd.dma_gather(
                            stag_sb[:, 2 * ch:2 * (ch + 1), :], fm3[:],
                            idx_sb[:, ch * cw:(ch + 1) * cw],
                            CHUNK, CHUNK, C, queue_num=(2 * t + ch) % NQ)
                ps = psump.tile([P, SB_PER_TILE, 512], mybir.dt.float32,
                                tag="ps")
                for j in range(SB_PER_TILE):
                    nc.tensor.matmul(
                        ps[:, j, 0:NPX],
                        stag_sb[0:KMAX, j, :],
                        w_sb[:, j, :],
                        start=True, stop=True,
                    )
                bounce = bouncep.tile([P, TPX], mybir.dt.bfloat16,
                                      tag="bounce")
                half = TPX // 2
                nc.scalar.copy(bounce[:, 0:half], ps[:, 0:2, 0:NPX])
                nc.vector.tensor_copy(bounce[:, half:TPX], ps[:, 2:4, 0:NPX])
                nc.scalar.dma_start(out[:, t * TPX:(t + 1) * TPX], bounce[:])
    nc.compile()
    return nc


def _get_nc():
    if "nc" not in _CACHE:
        _CACHE["nc"] = _build_nc()
    return _CACHE["nc"]


def _pack_core(idx, bary):
    """Greedy superblock packing of one core's face-sorted pixels.

    Returns (slot[131072] int64 global device slot per sorted-pixel-rank,
             idx_flat[nsb*128] int16 gather indices (3*face+v, padded),
             wmat [NTILES, KMAX, SB_PER_TILE, NPX] bf16)
    """
    bf16 = _bf16()
    bg = idx < 0
    idxc = np.where(bg, 0, idx).astype(np.int64)
    perm = np.argsort(idxc, kind="stable")       # pixel ids in face order
    sidx = idxc[perm]
    sbary = np.where(bg[perm][:, None], 0.0, bary[perm]).astype(np.float32)

    faces, counts = np.unique(sidx, return_counts=True)
    nsb_cap = NTILES * SB_PER_TILE

    # Walk face runs, assigning spans (face, u_local, sb, p0, cnt).
    span_sb = []
    span_u = []
    span_p0 = []
    span_cnt = []
    span_face = []
    sb = 0
    cur_px = 0
    cur_u = 0
    for f, cnt in zip(faces.tolist(), counts.tolist()):
        remaining = cnt
        first_in_sb = True
        u = -1
        while remaining > 0:
            if cur_px == NPX or (first_in_sb and cur_u == UMAX):
                sb += 1
                cur_px = 0
                cur_u = 0
                first_in_sb = True
            if first_in_sb:
                u = cur_u
                cur_u += 1
                first_in_sb = False
            take = min(remaining, NPX - cur_px)
            span_sb.append(sb)
            span_u.append(u)
            span_p0.append(cur_px)
            span_cnt.append(take)
            span_face.append(f)
            cur_px += take
            remaining -= take
            if remaining > 0:
                # face continues into the next superblock
                sb += 1
                cur_px = 0
                cur_u = 0
                first_in_sb = True
    nsb = sb + 1
    assert nsb <= nsb_cap, f"packing overflow: {nsb} > {nsb_cap}"

    span_sb = np.asarray(span_sb, dtype=np.int64)
    span_u = np.asarray(span_u, dtype=np.int64)
    span_p0 = np.asarray(span_p0, dtype=np.int64)
    span_cnt = np.asarray(span_cnt, dtype=np.int64)
    span_face = np.asarray(span_face, dtype=np.int64)

    # per-sorted-pixel: sb, u, p' via span expansion (spans are in sorted order)
    pix_sb = np.repeat(span_sb, span_cnt)
    pix_u = np.repeat(span_u, span_cnt)
    off_in_span = np.arange(len(sidx)) - np.repeat(
        np.cumsum(span_cnt) - span_cnt, span_cnt)
    pix_p = np.repeat(span_p0, span_cnt) + off_in_span
    slot = pix_sb * NPX + pix_p                      # global device column

    # gather index lists per superblock: row 3u+v -> 3*face+v
    sbu_face = np.zeros((nsb_cap, UMAX), dtype=np.int64)
    sbu_face[span_sb, span_u] = span_face

    idx128 = np.zeros((nsb_cap, P), dtype=np.int16)
    r = np.arange(KMAX)
    idx128[:, :KMAX] = (3 * sbu_face[:, r // 3] + (r % 3)).astype(np.int16)

    # weight matrix W[sb, 3u+v, p'] = bary_v  (exact; zero for bg/padding)
    w = np.zeros((nsb_cap, KMAX, NPX), dtype=np.float32)
    for v in range(3):
        w[pix_sb, 3 * pix_u + v, pix_p] = sbary[:, v]
    w = w.reshape(NTILES, SB_PER_TILE, KMAX, NPX).transpose(0, 2, 1, 3)
    w = np.ascontiguousarray(w).astype(bf16)

    # wrap idx streams for the gather (CHUNK=256 per call, 4 calls per tile)
    idx_flat = idx128.reshape(NTILES, NCHUNK, CHUNK)                # [nt,4,256]
    idxw = np.ascontiguousarray(
        idx_flat.reshape(NTILES, NCHUNK, CHUNK // 16, 16).transpose(0, 1, 3, 2))
    idxw = np.tile(idxw, (1, 1, 8, 1))              # [nt, 4, 128, 16]
    idxw = np.ascontiguousarray(
        idxw.transpose(0, 2, 1, 3).reshape(NTILES, P, NCHUNK * CHUNK // 16))

    return perm, slot, idxw, w


def _prep_in_maps(pix_to_face, bary_coords, face_memory):
    bf16 = _bf16()
    N, H, W_, K = pix_to_face.shape          # 4, 512, 512, 1
    assert (N, H, W_, K) == (4, 512, 512, 1)
    fm3 = np.asarray(face_memory, dtype=np.float32).reshape(3 * F, C)
    fm3 = fm3.astype(bf16)

    idx_all = np.asarray(pix_to_face).reshape(N, H, W_)
    bary_all = np.asarray(bary_coords, dtype=np.float32).reshape(N, H, W_, 3)

    in_maps = []
    maps = []
    for c in range(N_CORES):
        n, hh = c // 2, (c % 2) * 256
        idx = idx_all[n, hh:hh + 256].reshape(-1)
        bary = bary_all[n, hh:hh + 256].reshape(-1, 3)
        perm, slot, idxw, w = _pack_core(idx, bary)
        in_maps.append({"fm3": fm3, "idxw": idxw, "wmat": w})
        maps.append((perm, slot))
    _CACHE["maps"] = maps
    return in_maps


def _widen_bf16(a):
    u = np.asarray(a).view(np.uint16).astype(np.uint32) << 16
    return u.view(np.float32)


def _assemble(results, maps=None):
    maps = maps or _CACHE["maps"]
    out_full = np.empty((4, 128, 512, 512), dtype=np.float32)
    for c in range(N_CORES):
        n, hh = c // 2, (c % 2) * 256
        perm, slot = maps[c]
        dev = _widen_bf16(results[c]["out"])        # [128, NTILES*TPX]
        img = out_full[n, :, hh:hh + 256, :].reshape(128, NPIX_CORE)
        img[:, perm] = dev[:, slot]
    return out_full


def run(in_maps, trace=False, trace_kwargs=None):
    from concourse.bass_utils import run_bass_kernel_spmd

    nc = _get_nc()
    kw = {}
    if trace:
        kw = dict(trace=True, trace_kwargs=trace_kwargs or {})
    return run_bass_kernel_spmd(nc, in_maps, list(range(N_CORES)), **kw)


def kernel(pix_to_face, bary_coords, face_memory):
    in_maps = _prep_in_maps(pix_to_face, bary_coords, face_memory)
    res = run(in_maps)
    return _assemble(res.results)
